# revision 14
# baseline (speedup 1.0000x reference)
"""GIN-style GNN message-passing layer on 8 Trainium2 NeuronCores.

Math (per reference):
    m      = h[src] + edge_attr                       [E, 96]
    aggr   = segment_sum(m, dst, N)                   [N, 96]
    out    = (1+eps)*h + relu(aggr @ W1 + b1) @ W2 + b2

Distribution strategy (edge-parallel by dst ownership, zero collectives):
  Nodes are sorted by in-degree and grouped into 392 windows of 128 dst
  slots; window 8j+k runs as program step j on core k, so all 8 cores share
  one SPMD program whose per-step chunk count C_j (= max degree in the 8
  windows of that step, known at compile time) shrinks monotonically.

  The host materializes, per edge, the rows the device needs ("halo"
  sharding): for dst node at slot s with degree d, its d incoming edges
  occupy chunk columns 0..d-1 (h[src]) and C_j..C_j+d-1 (edge_attr) of
  slot s, zeros elsewhere. Segment-sum on device therefore degenerates to
  a plain sum over the 2*C_j chunk columns: a bf16 tree-fold (first level
  alternating DVE/GPSIMD, rest DVE) -- no gather DMA (the old SWDGE gather
  serialized ~880us on GPSIMD descriptor generation), no one-hot
  indicator, no scatter matmul.

  The stream is laid out partition-major (each SBUF partition's data for
  ALL windows is one contiguous HBM run) and loaded in multi-window
  grouped DMAs (~27KB contiguous per partition per transfer) alternating
  across both HWDGE queues (sync + scalar), since per-queue throughput is
  descriptor-rate limited. The MLP runs in bf16 on PE; b2 and the GIN
  residual (1+eps)*h are added by rank-1/diagonal matmuls accumulating
  into the same PSUM tile; ACT does relu + PSUM evacuation; outputs
  collect in a resident bf16 buffer flushed in 4 chunked DMAs (host
  upcasts to f32).
"""
import os
import numpy as np
import ml_dtypes

import concourse.bass as bass
import concourse.mybir as mybir
import concourse.tile as tile
from concourse import bacc
from concourse.bass_utils import run_bass_kernel_spmd
from concourse.masks import make_identity

# problem shape (hardcoded per contest contract)
N_NODES = 50000
N_EDGES = 800000
EMB = 96
HID = 192
P = 128
N_CORES = 8
W_PER_CORE = 49
N_WIN = W_PER_CORE * N_CORES          # 392 windows of 128 slots
N_SLOTS = N_WIN * P                   # 50176 >= N_NODES
F_GROUP_CAP = 144                     # max chunk columns per grouped DMA

LAST_RESULTS = None      # BassKernelResults of the most recent run (for test.py)
_PROGRAM_CACHE = {}


# ----------------------------------------------------------------- host plan
def _build_plan(src, dst):
    src = np.asarray(src).astype(np.int64)
    dst = np.asarray(dst).astype(np.int64)

    deg = np.bincount(dst, minlength=N_NODES)
    order = np.argsort(-deg, kind="stable")
    rank = np.empty(N_NODES, dtype=np.int64)
    rank[order] = np.arange(N_NODES)

    g_of_node = rank // P            # global window 0..391 (degree-sorted)
    slot_of_node = rank % P
    j_of_node = g_of_node // N_CORES  # program step
    k_of_node = g_of_node % N_CORES   # owning core

    deg_pad = np.zeros(N_SLOTS, dtype=np.int64)
    deg_pad[:N_NODES] = deg[order]
    # degree-sorted desc => window max = first element; step max = window 8j
    c_prog = np.maximum(deg_pad[np.arange(W_PER_CORE) * N_CORES * P], 1)

    # partition-major stream: row = slot * F_tot + off_j + chunk_col
    f_tot = int(2 * c_prog.sum())
    off = np.concatenate([[0], np.cumsum(2 * c_prog)])
    tot_rows = P * f_tot

    # chunk index of each edge = its rank among edges sharing the same dst
    eorder = np.argsort(dst, kind="stable")
    starts = np.searchsorted(dst[eorder], np.arange(N_NODES))
    chunk_of_e = np.empty(N_EDGES, dtype=np.int64)
    chunk_of_e[eorder] = np.arange(N_EDGES) - starts[dst[eorder]]

    vd = dst
    jd, kd, sd = j_of_node[vd], k_of_node[vd], slot_of_node[vd]
    cj = c_prog[jd]
    assert (chunk_of_e < cj).all()
    hrow = sd * f_tot + off[jd] + chunk_of_e
    arow = hrow + cj

    return dict(c_prog=c_prog, f_tot=f_tot, tot_rows=tot_rows, kd=kd,
                hrow=hrow, arow=arow, j_of_node=j_of_node,
                k_of_node=k_of_node, slot_of_node=slot_of_node)


def _make_groups(c_prog):
    """Greedy grouping of consecutive windows into DMA batches."""
    groups = []
    cur, cur_f = [], 0
    for w, c in enumerate(c_prog):
        f = 2 * int(c)
        if cur and cur_f + f > F_GROUP_CAP:
            groups.append(cur)
            cur, cur_f = [], 0
        cur.append(w)
        cur_f += f
    if cur:
        groups.append(cur)
    return groups


# -------------------------------------------------------------- device build
def _build_program(c_prog):
    c_prog = list(int(c) for c in c_prog)
    W = len(c_prog)
    f32 = mybir.dt.float32
    bf16 = mybir.dt.bfloat16
    off = np.concatenate([[0], np.cumsum([2 * c for c in c_prog])])
    f_tot = int(off[-1])
    groups = _make_groups(c_prog)
    f_group_max = max(int(off[g[-1] + 1] - off[g[0]]) for g in groups)
    flushes = {W // 4, W // 2, (3 * W) // 4, W}

    nc = bacc.Bacc("TRN2", target_bir_lowering=False, debug=False,
                   num_devices=N_CORES)
    t_stream = nc.dram_tensor("stream", [P * f_tot, EMB], bf16, kind="ExternalInput")
    t_hres = nc.dram_tensor("hres", [P * W, EMB], bf16, kind="ExternalInput")
    t_w1 = nc.dram_tensor("w1", [EMB, HID], bf16, kind="ExternalInput")
    t_b1 = nc.dram_tensor("b1", [HID, 1], f32, kind="ExternalInput")
    t_w2 = nc.dram_tensor("w2", [HID, EMB], bf16, kind="ExternalInput")
    t_b2r = nc.dram_tensor("b2r", [1, EMB], bf16, kind="ExternalInput")
    t_ones = nc.dram_tensor("ones", [1, P], bf16, kind="ExternalInput")
    t_epsb = nc.dram_tensor("epsb", [P, 1], f32, kind="ExternalInput")
    t_out = nc.dram_tensor("out", [P * W, EMB], bf16, kind="ExternalOutput")

    stream_v = t_stream[:].rearrange("(s f) e -> s f e", s=P)
    out_v = t_out[:].rearrange("(s j) e -> s j e", s=P)

    with tile.TileContext(nc) as tc:
        with (
            tc.tile_pool(name="const", bufs=1) as cpool,
            tc.tile_pool(name="work", bufs=4) as wpool,
            tc.tile_pool(name="small", bufs=4) as spool,
            tc.tile_pool(name="psumb", bufs=2, space="PSUM") as ppool_b,
            tc.tile_pool(name="psumc", bufs=2, space="PSUM") as ppool_c,
        ):
            ident = cpool.tile([P, P], bf16)
            make_identity(nc, ident[:])
            w1_t = cpool.tile([EMB, HID], bf16)
            nc.sync.dma_start(out=w1_t[:], in_=t_w1[:])
            w2a_t = cpool.tile([EMB, EMB], bf16)
            nc.sync.dma_start(out=w2a_t[:], in_=t_w2[0:EMB, :])
            w2b_t = cpool.tile([EMB, EMB], bf16)
            nc.sync.dma_start(out=w2b_t[:], in_=t_w2[EMB:HID, :])
            b1a = cpool.tile([EMB, 1], f32)
            nc.sync.dma_start(out=b1a[:], in_=t_b1[0:EMB, :])
            b1b = cpool.tile([EMB, 1], f32)
            nc.sync.dma_start(out=b1b[:], in_=t_b1[EMB:HID, :])
            b2r = cpool.tile([1, EMB], bf16)
            nc.sync.dma_start(out=b2r[:], in_=t_b2r[:])
            ones1 = cpool.tile([1, P], bf16)
            nc.sync.dma_start(out=ones1[:], in_=t_ones[:])
            scale = cpool.tile([P, 1], f32)
            nc.sync.dma_start(out=scale[:], in_=t_epsb[:])
            nc.vector.tensor_scalar_add(scale[:], scale[:], 1.0)
            # (1+eps) * identity: applies the GIN residual as one PSUM matmul
            scaled_i = cpool.tile([P, P], bf16)
            nc.vector.tensor_scalar(scaled_i[:], ident[:], scale[:, 0:1], None,
                                    op0=mybir.AluOpType.mult)

            # whole residual + output live in SBUF; host laid hres/out rows
            # as slot-major (row = slot*W + j) so these are single, fully
            # contiguous DMAs
            hres_all = cpool.tile([P, W, EMB], bf16)
            nc.sync.dma_start(
                out=hres_all[:],
                in_=t_hres[:].rearrange("(s j) e -> s j e", s=P))
            out_all = cpool.tile([P, W, EMB], bf16)

            dma_engines = [nc.sync, nc.scalar]
            prev_flush = 0
            for gi, grp in enumerate(groups):
                g0 = int(off[grp[0]])
                fg = int(off[grp[-1] + 1]) - g0
                st = wpool.tile([P, f_group_max, EMB], bf16, tag="st")
                dma_engines[gi % 2].dma_start(
                    out=st[:, 0:fg, :], in_=stream_v[:, g0:g0 + fg, :])

                for w in grp:
                    C = c_prog[w]
                    base = int(off[w]) - g0

                    # bf16 tree-fold of the 2C chunk columns (the segment-sum);
                    # the first (largest) fold level alternates DVE / GPSIMD
                    n = 2 * C
                    first = True
                    while n > 1:
                        m = n // 2
                        lo = n - 2 * m
                        eng = (nc.gpsimd if (first and w % 2 == 1) else nc.vector)
                        eng.tensor_tensor(
                            out=st[:, base + lo:base + lo + m, :],
                            in0=st[:, base + lo:base + lo + m, :],
                            in1=st[:, base + lo + m:base + n, :],
                            op=mybir.AluOpType.add)
                        n = lo + m
                        first = False

                    aggrT_p = ppool_c.tile([EMB, P], bf16, tag="aggrT")
                    nc.tensor.transpose(aggrT_p[:], st[:, base, :], ident[:])
                    aggrT_s = spool.tile([EMB, P], bf16, tag="aggrT_s")
                    nc.scalar.copy(aggrT_s[:], aggrT_p[:])

                    h1_p = ppool_c.tile([EMB, P], f32, tag="h1")
                    nc.tensor.matmul(h1_p[:], lhsT=w1_t[:, 0:EMB], rhs=aggrT_s[:],
                                     start=True, stop=True)
                    h2_p = ppool_c.tile([EMB, P], f32, tag="h2")
                    nc.tensor.matmul(h2_p[:], lhsT=w1_t[:, EMB:HID], rhs=aggrT_s[:],
                                     start=True, stop=True)
                    h1_s = spool.tile([EMB, P], bf16, tag="h1s")
                    nc.scalar.activation(h1_s[:], h1_p[:],
                                         mybir.ActivationFunctionType.Relu,
                                         bias=b1a[:])
                    h2_s = spool.tile([EMB, P], bf16, tag="h2s")
                    nc.scalar.activation(h2_s[:], h2_p[:],
                                         mybir.ActivationFunctionType.Relu,
                                         bias=b1b[:])

                    out_p = ppool_b.tile([P, EMB], f32, tag="outp")
                    nc.tensor.matmul(out_p[:], lhsT=ones1[:], rhs=b2r[:],
                                     start=True, stop=False)
                    nc.tensor.matmul(out_p[:], lhsT=scaled_i[:],
                                     rhs=hres_all[:, w, :],
                                     start=False, stop=False)
                    nc.tensor.matmul(out_p[:], lhsT=h1_s[:], rhs=w2a_t[:],
                                     start=False, stop=False)
                    nc.tensor.matmul(out_p[:], lhsT=h2_s[:], rhs=w2b_t[:],
                                     start=False, stop=True)
                    nc.scalar.copy(out_all[:, w, :], out_p[:])

                    if w + 1 in flushes:
                        nc.sync.dma_start(
                            out=out_v[:, prev_flush:w + 1, :],
                            in_=out_all[:, prev_flush:w + 1, :])
                        prev_flush = w + 1

    nc.compile()
    return nc


# ------------------------------------------------------------------- kernel
def kernel(h, edge_attr, src, dst, W1, b1, W2, b2, eps):
    global LAST_RESULTS
    h = np.asarray(h, dtype=np.float32)
    edge_attr = np.asarray(edge_attr, dtype=np.float32)
    W1 = np.asarray(W1, dtype=np.float32)
    b1 = np.asarray(b1, dtype=np.float32)
    W2 = np.asarray(W2, dtype=np.float32)
    b2 = np.asarray(b2, dtype=np.float32)
    eps = np.asarray(eps, dtype=np.float32)

    plan = _build_plan(src, dst)
    c_prog = plan["c_prog"]
    tot_rows = plan["tot_rows"]

    key = tuple(int(c) for c in c_prog)
    if key not in _PROGRAM_CACHE:
        _PROGRAM_CACHE[key] = _build_program(c_prog)
    nc = _PROGRAM_CACHE[key]

    # ---- per-slot host arrays (halo-shard h[src] and edge_attr per core) ----
    h_bf = h.astype(ml_dtypes.bfloat16)
    ea_bf = edge_attr.astype(ml_dtypes.bfloat16)
    stream = np.zeros((N_CORES, tot_rows, EMB), dtype=ml_dtypes.bfloat16)
    kd, hrow, arow = plan["kd"], plan["hrow"], plan["arow"]
    src64 = np.asarray(src).astype(np.int64)
    stream[kd, hrow] = h_bf[src64]
    stream[kd, arow] = ea_bf

    # residual/output shard rows are slot-major: row = slot*W + j
    hres = np.zeros((N_CORES, P * W_PER_CORE, EMB), dtype=ml_dtypes.bfloat16)
    shard_row = plan["slot_of_node"] * W_PER_CORE + plan["j_of_node"]
    hres[plan["k_of_node"], shard_row] = h_bf

    b2r = b2[None, :].astype(ml_dtypes.bfloat16)
    ones = np.ones((1, P), dtype=ml_dtypes.bfloat16)
    epsb = np.full((P, 1), eps[0], dtype=np.float32)
    w1_bf = W1.astype(ml_dtypes.bfloat16)
    w2_bf = W2.astype(ml_dtypes.bfloat16)

    in_maps = []
    for k in range(N_CORES):
        in_maps.append(dict(
            stream=stream[k], hres=hres[k],
            w1=w1_bf, b1=b1[:, None], w2=w2_bf, b2r=b2r, ones=ones, epsb=epsb))

    LAST_RESULTS = run_bass_kernel_spmd(nc, in_maps, core_ids=list(range(N_CORES)),
                                        tmpdir=os.environ.get("GNN_TRACE_DIR") or None)
    shards = np.stack([np.asarray(LAST_RESULTS.results[k]["out"],
                                  dtype=np.float32) for k in range(N_CORES)])
    out = shards[plan["k_of_node"], shard_row]
    return np.ascontiguousarray(out, dtype=np.float32)


# revision 15
# speedup vs baseline: 1.3482x; 1.3482x over previous
"""GIN-style GNN message-passing layer on 8 Trainium2 NeuronCores.

Math (per reference):
    m      = h[src] + edge_attr                       [E, 96]
    aggr   = segment_sum(m, dst, N)                   [N, 96]
    out    = (1+eps)*h + relu(aggr @ W1 + b1) @ W2 + b2

Distribution strategy (edge-parallel by dst ownership, zero collectives):
  Nodes are sorted by in-degree and grouped into 392 windows of 128 dst
  slots; window 8j+k runs as program step j on core k, so all 8 cores share
  one SPMD program whose per-step chunk count C_j (= max degree in the 8
  windows of that step, known at compile time) shrinks monotonically.

  The host materializes, per edge, the rows the device needs ("halo"
  sharding): for dst node at slot s with degree d, its d incoming edges
  occupy chunk columns 0..d-1 (h[src]) and C_j..C_j+d-1 (edge_attr) of
  slot s, zeros elsewhere. Segment-sum on device therefore degenerates to
  a plain sum over the 2*C_j chunk columns: a bf16 tree-fold (first level
  alternating DVE/GPSIMD, rest DVE) -- no gather DMA (the old SWDGE gather
  serialized ~880us on GPSIMD descriptor generation), no one-hot
  indicator, no scatter matmul.

  The stream is laid out partition-major (each SBUF partition's data for
  ALL windows is one contiguous HBM run) and loaded in multi-window
  grouped DMAs (~27KB contiguous per partition per transfer) alternating
  across both HWDGE queues (sync + scalar), since per-queue throughput is
  descriptor-rate limited. The MLP runs in bf16 on PE; b2 and the GIN
  residual (1+eps)*h are added by rank-1/diagonal matmuls accumulating
  into the same PSUM tile; ACT does relu + PSUM evacuation; outputs
  collect in a resident bf16 buffer flushed in 4 chunked DMAs (host
  upcasts to f32).
"""
import os
import numpy as np
import ml_dtypes

import concourse.bass as bass
import concourse.mybir as mybir
import concourse.tile as tile
from concourse import bacc
from concourse.bass_utils import run_bass_kernel_spmd
from concourse.masks import make_identity

# problem shape (hardcoded per contest contract)
N_NODES = 50000
N_EDGES = 800000
EMB = 96
HID = 192
P = 128
N_CORES = 8
W_PER_CORE = 49
N_WIN = W_PER_CORE * N_CORES          # 392 windows of 128 slots
N_SLOTS = N_WIN * P                   # 50176 >= N_NODES
F_GROUP_CAP = 144                     # max chunk columns per grouped DMA

LAST_RESULTS = None      # BassKernelResults of the most recent run (for test.py)
_PROGRAM_CACHE = {}


# ----------------------------------------------------------------- host plan
def _build_plan(src, dst):
    src = np.asarray(src).astype(np.int64)
    dst = np.asarray(dst).astype(np.int64)

    deg = np.bincount(dst, minlength=N_NODES)
    order = np.argsort(-deg, kind="stable")
    rank = np.empty(N_NODES, dtype=np.int64)
    rank[order] = np.arange(N_NODES)

    g_of_node = rank // P            # global window 0..391 (degree-sorted)
    slot_of_node = rank % P
    j_of_node = g_of_node // N_CORES  # program step
    k_of_node = g_of_node % N_CORES   # owning core

    deg_pad = np.zeros(N_SLOTS, dtype=np.int64)
    deg_pad[:N_NODES] = deg[order]
    # degree-sorted desc => window max = first element; step max = window 8j
    c_prog = np.maximum(deg_pad[np.arange(W_PER_CORE) * N_CORES * P], 1)

    # partition-major stream: row = slot * F_tot + off_j + chunk_col
    f_tot = int(2 * c_prog.sum())
    off = np.concatenate([[0], np.cumsum(2 * c_prog)])
    tot_rows = P * f_tot

    # chunk index of each edge = its rank among edges sharing the same dst
    eorder = np.argsort(dst, kind="stable")
    starts = np.searchsorted(dst[eorder], np.arange(N_NODES))
    chunk_of_e = np.empty(N_EDGES, dtype=np.int64)
    chunk_of_e[eorder] = np.arange(N_EDGES) - starts[dst[eorder]]

    vd = dst
    jd, kd, sd = j_of_node[vd], k_of_node[vd], slot_of_node[vd]
    cj = c_prog[jd]
    assert (chunk_of_e < cj).all()
    hrow = sd * f_tot + off[jd] + chunk_of_e
    arow = hrow + cj

    return dict(c_prog=c_prog, f_tot=f_tot, tot_rows=tot_rows, kd=kd,
                hrow=hrow, arow=arow, j_of_node=j_of_node,
                k_of_node=k_of_node, slot_of_node=slot_of_node)


def _make_groups(c_prog):
    """Greedy grouping of consecutive windows into DMA batches."""
    groups = []
    cur, cur_f = [], 0
    for w, c in enumerate(c_prog):
        f = 2 * int(c)
        if cur and cur_f + f > F_GROUP_CAP:
            groups.append(cur)
            cur, cur_f = [], 0
        cur.append(w)
        cur_f += f
    if cur:
        groups.append(cur)
    return groups


# -------------------------------------------------------------- device build
def _build_program(c_prog):
    c_prog = list(int(c) for c in c_prog)
    W = len(c_prog)
    f32 = mybir.dt.float32
    bf16 = mybir.dt.bfloat16
    off = np.concatenate([[0], np.cumsum([2 * c for c in c_prog])])
    f_tot = int(off[-1])
    groups = _make_groups(c_prog)
    f_group_max = max(int(off[g[-1] + 1] - off[g[0]]) for g in groups)
    flushes = {W // 4, W // 2, (3 * W) // 4, W}

    nc = bacc.Bacc("TRN2", target_bir_lowering=False, debug=False,
                   num_devices=N_CORES)
    t_stream = nc.dram_tensor("stream", [P * f_tot, EMB], bf16, kind="ExternalInput")
    t_hres = nc.dram_tensor("hres", [P * W, EMB], bf16, kind="ExternalInput")
    t_w1 = nc.dram_tensor("w1", [EMB, HID], bf16, kind="ExternalInput")
    t_b1 = nc.dram_tensor("b1", [HID, 1], f32, kind="ExternalInput")
    t_w2 = nc.dram_tensor("w2", [HID, EMB], bf16, kind="ExternalInput")
    t_b2r = nc.dram_tensor("b2r", [1, EMB], bf16, kind="ExternalInput")
    t_ones = nc.dram_tensor("ones", [1, P], bf16, kind="ExternalInput")
    t_epsb = nc.dram_tensor("epsb", [P, 1], f32, kind="ExternalInput")
    t_out = nc.dram_tensor("out", [P * W, EMB], bf16, kind="ExternalOutput")

    stream_v = t_stream[:].rearrange("(s f) e -> s f e", s=P)
    out_v = t_out[:].rearrange("(s j) e -> s j e", s=P)

    with tile.TileContext(nc) as tc:
        with (
            tc.tile_pool(name="const", bufs=1) as cpool,
            tc.tile_pool(name="work", bufs=4) as wpool,
            tc.tile_pool(name="small", bufs=4) as spool,
            tc.tile_pool(name="psumb", bufs=2, space="PSUM") as ppool_b,
            tc.tile_pool(name="psumc", bufs=2, space="PSUM") as ppool_c,
        ):
            ident = cpool.tile([P, P], bf16)
            make_identity(nc, ident[:])
            w1_t = cpool.tile([EMB, HID], bf16)
            nc.sync.dma_start(out=w1_t[:], in_=t_w1[:])
            w2a_t = cpool.tile([EMB, EMB], bf16)
            nc.sync.dma_start(out=w2a_t[:], in_=t_w2[0:EMB, :])
            w2b_t = cpool.tile([EMB, EMB], bf16)
            nc.sync.dma_start(out=w2b_t[:], in_=t_w2[EMB:HID, :])
            b1a = cpool.tile([EMB, 1], f32)
            nc.sync.dma_start(out=b1a[:], in_=t_b1[0:EMB, :])
            b1b = cpool.tile([EMB, 1], f32)
            nc.sync.dma_start(out=b1b[:], in_=t_b1[EMB:HID, :])
            b2r = cpool.tile([1, EMB], bf16)
            nc.sync.dma_start(out=b2r[:], in_=t_b2r[:])
            ones1 = cpool.tile([1, P], bf16)
            nc.sync.dma_start(out=ones1[:], in_=t_ones[:])
            scale = cpool.tile([P, 1], f32)
            nc.sync.dma_start(out=scale[:], in_=t_epsb[:])
            nc.vector.tensor_scalar_add(scale[:], scale[:], 1.0)
            # (1+eps) * identity: applies the GIN residual as one PSUM matmul
            scaled_i = cpool.tile([P, P], bf16)
            nc.vector.tensor_scalar(scaled_i[:], ident[:], scale[:, 0:1], None,
                                    op0=mybir.AluOpType.mult)

            # whole residual + output live in SBUF; host laid hres/out rows
            # as slot-major (row = slot*W + j) so these are single, fully
            # contiguous DMAs
            hres_all = cpool.tile([P, W, EMB], bf16)
            nc.sync.dma_start(
                out=hres_all[:],
                in_=t_hres[:].rearrange("(s j) e -> s j e", s=P))
            out_all = cpool.tile([P, W, EMB], bf16)

            # three independent DMA queue streams (~105GB/s each): the two
            # HWDGE engines plus the gpsimd SWDGE queue as a plain copier
            dma_engines = [nc.sync, nc.scalar, nc.gpsimd]
            prev_flush = 0
            for gi, grp in enumerate(groups):
                g0 = int(off[grp[0]])
                fg = int(off[grp[-1] + 1]) - g0
                st = wpool.tile([P, f_group_max, EMB], bf16, tag="st")
                dma_engines[gi % 3].dma_start(
                    out=st[:, 0:fg, :], in_=stream_v[:, g0:g0 + fg, :])

                for w in grp:
                    C = c_prog[w]
                    base = int(off[w]) - g0

                    # bf16 tree-fold of the 2C chunk columns (the segment-sum)
                    n = 2 * C
                    while n > 1:
                        m = n // 2
                        lo = n - 2 * m
                        nc.vector.tensor_tensor(
                            out=st[:, base + lo:base + lo + m, :],
                            in0=st[:, base + lo:base + lo + m, :],
                            in1=st[:, base + lo + m:base + n, :],
                            op=mybir.AluOpType.add)
                        n = lo + m

                    aggrT_p = ppool_c.tile([EMB, P], bf16, tag="aggrT")
                    nc.tensor.transpose(aggrT_p[:], st[:, base, :], ident[:])
                    aggrT_s = spool.tile([EMB, P], bf16, tag="aggrT_s")
                    nc.scalar.copy(aggrT_s[:], aggrT_p[:])

                    h1_p = ppool_c.tile([EMB, P], f32, tag="h1")
                    nc.tensor.matmul(h1_p[:], lhsT=w1_t[:, 0:EMB], rhs=aggrT_s[:],
                                     start=True, stop=True)
                    h2_p = ppool_c.tile([EMB, P], f32, tag="h2")
                    nc.tensor.matmul(h2_p[:], lhsT=w1_t[:, EMB:HID], rhs=aggrT_s[:],
                                     start=True, stop=True)
                    h1_s = spool.tile([EMB, P], bf16, tag="h1s")
                    nc.scalar.activation(h1_s[:], h1_p[:],
                                         mybir.ActivationFunctionType.Relu,
                                         bias=b1a[:])
                    h2_s = spool.tile([EMB, P], bf16, tag="h2s")
                    nc.scalar.activation(h2_s[:], h2_p[:],
                                         mybir.ActivationFunctionType.Relu,
                                         bias=b1b[:])

                    out_p = ppool_b.tile([P, EMB], f32, tag="outp")
                    nc.tensor.matmul(out_p[:], lhsT=ones1[:], rhs=b2r[:],
                                     start=True, stop=False)
                    nc.tensor.matmul(out_p[:], lhsT=scaled_i[:],
                                     rhs=hres_all[:, w, :],
                                     start=False, stop=False)
                    nc.tensor.matmul(out_p[:], lhsT=h1_s[:], rhs=w2a_t[:],
                                     start=False, stop=False)
                    nc.tensor.matmul(out_p[:], lhsT=h2_s[:], rhs=w2b_t[:],
                                     start=False, stop=True)
                    nc.scalar.copy(out_all[:, w, :], out_p[:])

                    if w + 1 in flushes:
                        nc.sync.dma_start(
                            out=out_v[:, prev_flush:w + 1, :],
                            in_=out_all[:, prev_flush:w + 1, :])
                        prev_flush = w + 1

    nc.compile()
    return nc


# ------------------------------------------------------------------- kernel
def kernel(h, edge_attr, src, dst, W1, b1, W2, b2, eps):
    global LAST_RESULTS
    h = np.asarray(h, dtype=np.float32)
    edge_attr = np.asarray(edge_attr, dtype=np.float32)
    W1 = np.asarray(W1, dtype=np.float32)
    b1 = np.asarray(b1, dtype=np.float32)
    W2 = np.asarray(W2, dtype=np.float32)
    b2 = np.asarray(b2, dtype=np.float32)
    eps = np.asarray(eps, dtype=np.float32)

    plan = _build_plan(src, dst)
    c_prog = plan["c_prog"]
    tot_rows = plan["tot_rows"]

    key = tuple(int(c) for c in c_prog)
    if key not in _PROGRAM_CACHE:
        _PROGRAM_CACHE[key] = _build_program(c_prog)
    nc = _PROGRAM_CACHE[key]

    # ---- per-slot host arrays (halo-shard h[src] and edge_attr per core) ----
    h_bf = h.astype(ml_dtypes.bfloat16)
    ea_bf = edge_attr.astype(ml_dtypes.bfloat16)
    stream = np.zeros((N_CORES, tot_rows, EMB), dtype=ml_dtypes.bfloat16)
    kd, hrow, arow = plan["kd"], plan["hrow"], plan["arow"]
    src64 = np.asarray(src).astype(np.int64)
    stream[kd, hrow] = h_bf[src64]
    stream[kd, arow] = ea_bf

    # residual/output shard rows are slot-major: row = slot*W + j
    hres = np.zeros((N_CORES, P * W_PER_CORE, EMB), dtype=ml_dtypes.bfloat16)
    shard_row = plan["slot_of_node"] * W_PER_CORE + plan["j_of_node"]
    hres[plan["k_of_node"], shard_row] = h_bf

    b2r = b2[None, :].astype(ml_dtypes.bfloat16)
    ones = np.ones((1, P), dtype=ml_dtypes.bfloat16)
    epsb = np.full((P, 1), eps[0], dtype=np.float32)
    w1_bf = W1.astype(ml_dtypes.bfloat16)
    w2_bf = W2.astype(ml_dtypes.bfloat16)

    in_maps = []
    for k in range(N_CORES):
        in_maps.append(dict(
            stream=stream[k], hres=hres[k],
            w1=w1_bf, b1=b1[:, None], w2=w2_bf, b2r=b2r, ones=ones, epsb=epsb))

    LAST_RESULTS = run_bass_kernel_spmd(nc, in_maps, core_ids=list(range(N_CORES)),
                                        tmpdir=os.environ.get("GNN_TRACE_DIR") or None)
    shards = np.stack([np.asarray(LAST_RESULTS.results[k]["out"],
                                  dtype=np.float32) for k in range(N_CORES)])
    out = shards[plan["k_of_node"], shard_row]
    return np.ascontiguousarray(out, dtype=np.float32)
